# revision 1
# baseline (speedup 1.0000x reference)
"""Trainium2 Bass kernel for nn_AuxLoss (aux CE loss + erf regularizer, segment-
mean over K=10 classes), data-parallel over 8 NeuronCores.

Math (per reference):
  f(u)      = 0.5 - 0.5*erf((-0.5 - u)/(sigma*sqrt2)) = 0.5 + 0.5*erf((u+0.5)*sqrt2)
  row_reg_n = sum_d f(u[n,d])
  row_ce_n  = logsumexp(yg[n,:]) - yg[n, yhat[n]]
  per-class means over rows with yhat==k, averaged over present classes:
  out = mean_k(seg_ce/cnt) + lmbd * mean_k(seg_reg/(cnt*D))

Device strategy per core (131072 rows):
  - partition p holds a contiguous slab of 1024 rows -> fully contiguous DMAs
    (u on the sync HWDGE ring, yg/yhat on the gpsimd SWDGE ring)
  - 16 chunks of w=64 rows/partition; ACT functions are batched over groups of
    chunks (erf xG, exp xG, ln xG, tapered at the end) and explicitly
    order-chained to amortize ACT table-set loads; tables prewarmed at t~0
  - per chunk one combined bf16 "work" tile [128, w, 75]:
      cols 0:64  erf(sqrt2*u + sqrt2/2)        (ACT, strided out; the 0.5+0.5*
                 affine is folded into the final fixup: seg_f = 0.5*D*cnt + 0.5*seg_erf)
      cols 64:74 onehot*yg                      (DVE; column-sum of this block's
                 segment-matmul = seg of yg[n,yhat[n]] -- diagonal trick)
      col  74    ones                           (counts)
    onehot[p,r,c] = (yhat==c) via iota compare (DVE, bf16); exp runs in-place
    on the yg tile; lse = ln(sumexp) feeds a DVE class-major masked reduce into
    an SBUF accumulator (so the PE stream never waits on ln) finished by one
    ones-stationary matmul
  - PE: per 128-row group g one matmul: onehot[:,g,:] stationary (128x10),
    work[:,g,:] moving (128x75), accumulating PSUM [10,75] over all 1024 groups
  - local pre-reduce to [10,4] (erf_sum, picked_sum, count, lse_sum), 160 B
    AllGather across 8 cores + local sum (a warm-up collective at t~0 absorbs
    the collective-stream entry barrier; its readback is pinned to the stream
    end to avoid DMA-queue head-of-line blocking), final means on-device.
"""

import math
import sys

if "/opt/trn_rl_repo" not in sys.path:
    sys.path.insert(0, "/opt/trn_rl_repo")

import numpy as np

N_CORES = 8
N_FULL = 1048576
C = 10
D = 64
P = 128
ROWS_PER_CORE = N_FULL // N_CORES  # 131072
SQ2 = math.sqrt(2.0)
W_COLS = D + C + 1  # erf block | onehot*yg block | ones


def build(rows_per_core=ROWS_PER_CORE, w=64, act_batch=4):
    """Build + compile the 8-core Bacc graph. w = rows/partition/chunk."""
    from concourse import bacc, mybir, tile

    f32 = mybir.dt.float32
    bf16 = mybir.dt.bfloat16
    i32 = mybir.dt.int32
    FT = mybir.ActivationFunctionType
    ALU = mybir.AluOpType
    AX = mybir.AxisListType

    rpp = rows_per_core // P
    assert rpp * P == rows_per_core
    nch = rpp // w
    assert nch * w == rpp
    sched = []
    rem = nch
    while rem > 0:
        step = act_batch if rem > 2 * act_batch else max(rem // 2, 1)
        step = min(step, rem)
        sched.append(step)
        rem -= step
    starts = [sum(sched[:i]) for i in range(len(sched))]
    nbatch = len(sched)

    nc = bacc.Bacc("TRN2", target_bir_lowering=False, debug=False, num_devices=N_CORES)

    yh_d = nc.dram_tensor("yhat", [rows_per_core], i32, kind="ExternalInput")
    yg_d = nc.dram_tensor("yg", [rows_per_core, C], f32, kind="ExternalInput")
    u_d = nc.dram_tensor("u_zg", [rows_per_core, D], f32, kind="ExternalInput")
    lm_d = nc.dram_tensor("lmbd", [1, 1], f32, kind="ExternalInput")
    out_d = nc.dram_tensor("out", [1, 1], f32, kind="ExternalOutput")
    warm_in = nc.dram_tensor("warm_in", [1, 1], f32)
    warm_out = nc.dram_tensor("warm_out", [1, 1], f32, addr_space="Shared")
    cc_in = nc.dram_tensor("cc_in", [C, 4], f32)
    cc_out = nc.dram_tensor("cc_out", [N_CORES, C, 4], f32, addr_space="Shared")

    u_v = u_d[:].rearrange("(p r) d -> p r d", p=P)
    yg_v = yg_d[:].rearrange("(p r) c -> p r c", p=P)
    yh_v = yh_d[:].rearrange("(p r) -> p r", p=P)

    rg = [list(range(N_CORES))]

    from concourse.tile_rust import add_dep_helper

    last_act = [None]

    def act_ordered(*args, **kwargs):
        """scalar.activation with an explicit chain dep so the Tile scheduler
        cannot interleave ACT functions (each interleave costs a ~1.3 us
        ACT table-set load)."""
        inst = nc.scalar.activation(*args, **kwargs)
        raw = getattr(inst, "ins", inst)
        if last_act[0] is not None:
            add_dep_helper(raw, last_act[0], sync=True, reason="act set order")
        last_act[0] = raw
        return inst

    with tile.TileContext(nc) as tc:
        with (
            tc.tile_pool(name="const", bufs=1) as constp,
            tc.tile_pool(name="io", bufs=1) as iop,
            tc.tile_pool(name="work", bufs=1) as workp,
            tc.tile_pool(name="psum", bufs=1, space="PSUM") as psump,
            tc.tile_pool(name="fin", bufs=1) as finp,
        ):
            # --- warm-up collective: absorb entry barrier early, overlapped
            # with the main DMA/compute stream ---
            wz = constp.tile([1, 1], f32)
            nc.vector.memset(wz[:], 0.0)
            nc.gpsimd.dma_start(warm_in[:], wz[:])
            nc.gpsimd.collective_compute(
                "AllReduce", ALU.add, replica_groups=rg,
                ins=[warm_in[:].opt()], outs=[warm_out[:].opt()],
            )
            wres = constp.tile([1, 1], f32)

            # --- constants ---
            erf_bias = constp.tile([P, 1], f32)
            nc.vector.memset(erf_bias[:], 0.5 * SQ2)
            iota_f = constp.tile([P, 1, C], f32)
            nc.gpsimd.iota(
                iota_f[:, 0, :], [[1, C]],
                channel_multiplier=0, allow_small_or_imprecise_dtypes=True,
            )
            yh_i = constp.tile([P, rpp], i32)
            nc.gpsimd.dma_start(yh_i[:], yh_v)
            yh_f = constp.tile([P, rpp], f32)
            nc.vector.tensor_copy(yh_f[:], yh_i[:])

            # prewarm the three ACT table sets while the first DMAs fly
            warm_act = constp.tile([1, 1], f32)
            nc.vector.memset(warm_act[:], 1.0)
            wa_o = constp.tile([1, 1], f32)
            act_ordered(wa_o[:], warm_act[:], FT.Ln)
            act_ordered(wa_o[:], warm_act[:], FT.Exp)
            act_ordered(wa_o[:], warm_act[:], FT.Erf, bias=erf_bias[0:1, :], scale=SQ2)

            ones128 = constp.tile([P, 1], f32)
            nc.vector.memset(ones128[:], 1.0)
            lacc = constp.tile([P, C], f32)
            nc.vector.memset(lacc[:], 0.0)

            ps = psump.tile([C, W_COLS], f32)
            ps_l = psump.tile([C, 1], f32)

            def chunks_of(b):
                return range(starts[b], starts[b] + sched[b])

            u_ts, yg_ts, work_ts, oh_ts, sume_ts = {}, {}, {}, {}, {}

            for b in range(nbatch):
                # DMAs: yg on the gpsimd (SWDGE) ring; u in 2-chunk 4 MB pairs
                # alternating between the sync (HWDGE) and gpsimd (SWDGE) paths
                for ci in chunks_of(b):
                    r0, r1 = ci * w, (ci + 1) * w
                    yg_t = iop.tile([P, w, C], f32, name="yg_t", bufs=2 * act_batch)
                    nc.gpsimd.dma_start(yg_t[:], yg_v[:, r0:r1, :])
                    yg_ts[ci] = yg_t
                    u_t = iop.tile([P, w, D], f32, name="u_t", bufs=5)
                    nc.sync.dma_start(u_t[:], u_v[:, r0:r1, :])
                    u_ts[ci] = u_t[:]

                # erf batch (one ACT table load)
                for ci in chunks_of(b):
                    work_t = workp.tile(
                        [P, w, W_COLS], bf16, name="work_t", bufs=2 * act_batch
                    )
                    act_ordered(
                        work_t[:, :, 0:D], u_ts.pop(ci), FT.Erf,
                        bias=erf_bias[:], scale=SQ2,
                    )
                    work_ts[ci] = work_t

                # DVE: onehot + onehot*yg + ones while ACT works
                for ci in chunks_of(b):
                    r0, r1 = ci * w, (ci + 1) * w
                    oh_t = workp.tile([P, w, C], bf16, name="oh_t", bufs=2 * act_batch)
                    nc.vector.tensor_tensor(
                        oh_t[:],
                        yh_f[:, r0:r1].broadcast_to([P, w, C]),
                        iota_f[:].broadcast_to([P, w, C]),
                        ALU.is_equal,
                    )
                    oh_ts[ci] = oh_t
                    work_t = work_ts[ci]
                    nc.vector.tensor_tensor(
                        work_t[:, :, D : D + C], oh_t[:], yg_ts[ci][:], ALU.mult
                    )
                    nc.vector.memset(work_t[:, :, D + C], 1.0)

                # PE: one matmul per 128-row group (gated only by erf + DVE)
                for ci in chunks_of(b):
                    work_t = work_ts.pop(ci)
                    oh_t = oh_ts[ci]
                    for g in range(w):
                        first = ci == 0 and g == 0
                        last = ci == nch - 1 and g == w - 1
                        nc.tensor.matmul(
                            ps[:], oh_t[:, g, :], work_t[:, g, :],
                            start=first, stop=last,
                        )

                # exp batch (one load), in-place into yg; row-sums on DVE
                for ci in chunks_of(b):
                    yg_t = yg_ts.pop(ci)
                    act_ordered(yg_t[:], yg_t[:], FT.Exp)
                    sume_t = workp.tile([P, w], f32, name="sume_t", bufs=act_batch + 1)
                    nc.vector.reduce_sum(sume_t[:], yg_t[:], axis=AX.X)
                    sume_ts[ci] = sume_t

                # ln batch (one load); lse segment-sum via DVE (class-major
                # masked blocked reduce) so the PE stream never waits on ln
                for ci in chunks_of(b):
                    lse_t = workp.tile([P, w], f32, name="lse_t", bufs=3)
                    act_ordered(lse_t[:], sume_ts.pop(ci)[:], FT.Ln)
                    oh_t = oh_ts.pop(ci)
                    oh_cm = oh_t[:].transpose([0, 2, 1])
                    ltmp = workp.tile([P, C, w], bf16, name="ltmp", bufs=2)
                    nc.vector.tensor_tensor(
                        ltmp[:], oh_cm,
                        lse_t[:].broadcast_to([P, w, C]).transpose([0, 2, 1]),
                        ALU.mult,
                    )
                    lred = workp.tile([P, C], f32, name="lred", bufs=2)
                    nc.vector.reduce_sum(lred[:], ltmp[:], axis=AX.X)
                    nc.vector.tensor_tensor(lacc[:], lacc[:], lred[:], ALU.add)

            # partition-reduce the lse accumulator: lacc.T @ ones lands the
            # [10,1] result on 10 partitions, matching the psum layout
            nc.tensor.matmul(ps_l[:], lacc[:], ones128[:], start=True, stop=True)

            # --- local pre-reduce -> [10, 4] (erf, picked, counts, lse) ---
            acc = finp.tile([C, 4], f32)
            nc.vector.reduce_sum(acc[:, 0:1], ps[:, 0:D], axis=AX.X)
            nc.vector.reduce_sum(acc[:, 1:2], ps[:, D : D + C], axis=AX.X)
            nc.vector.tensor_copy(acc[:, 2:3], ps[:, D + C : D + C + 1])
            nc.vector.tensor_copy(acc[:, 3:4], ps_l[:])
            nc.sync.dma_start(cc_in[:], acc[:])
            nc.gpsimd.collective_compute(
                "AllGather", ALU.bypass, replica_groups=rg,
                ins=[cc_in[:].opt()], outs=[cc_out[:].opt()],
            )
            # warm-up readback issued HERE (end of stream) with an explicit dep
            # so the scheduler cannot hoist it: if it lands early in the gpsimd
            # FIFO, its wait on the warm collective head-of-line-blocks every
            # yg DMA behind it for ~60 us.
            wres_dma = nc.gpsimd.dma_start(wres[:], warm_out[:])
            add_dep_helper(
                getattr(wres_dma, "ins", wres_dma), last_act[0],
                sync=True, reason="keep warm-up readback at stream end",
            )

            # --- gather-sum + final per-class means on partition 0 ---
            fing = finp.tile([1, N_CORES, C * 4], f32)
            nc.sync.dma_start(
                fing[:],
                cc_out[:].rearrange("(o n) k c -> o n (k c)", o=1),
            )
            finv = finp.tile([1, C * 4], f32)
            nc.vector.tensor_tensor(
                finv[:], fing[:, 0, :], fing[:, 1, :], ALU.add
            )
            for n in range(2, N_CORES):
                nc.vector.tensor_tensor(
                    finv[:], finv[:], fing[:, n, :], ALU.add
                )
            f3 = finv[:].rearrange("p (k c) -> p k c", k=C)  # [1, 10, 4]
            e10 = f3[:, :, 0]  # strided [1, 10] APs
            p10 = f3[:, :, 1]
            cn10 = f3[:, :, 2]
            l10 = f3[:, :, 3]

            ce10 = finp.tile([1, C], f32)  # seg_ce = seg_lse - seg_picked
            nc.vector.tensor_tensor(ce10[:], l10, p10, ALU.subtract)
            mask = finp.tile([1, C], f32)
            nc.vector.tensor_scalar(mask[:], cn10, 0.0, None, ALU.is_gt)
            one_m = finp.tile([1, C], f32)
            nc.vector.tensor_scalar(one_m[:], mask[:], -1.0, 1.0, ALU.mult, ALU.add)
            den = finp.tile([1, C], f32)
            nc.vector.tensor_tensor(den[:], one_m[:], cn10, ALU.add)
            rinv = finp.tile([1, C], f32)
            nc.vector.reciprocal(rinv[:], den[:])

            # reg_c = (0.5*cnt + seg_erf/(2D)) / cnt ; aux_c = seg_ce / cnt
            hc = finp.tile([1, C], f32)
            nc.vector.tensor_scalar(hc[:], cn10, 0.5, None, ALU.mult)
            rnum = finp.tile([1, C], f32)
            nc.vector.scalar_tensor_tensor(
                rnum[:], e10, 1.0 / (2.0 * D), hc[:], ALU.mult, ALU.add
            )
            regc = finp.tile([1, C], f32)
            nc.vector.tensor_mul(regc[:], rnum[:], rinv[:])
            auxc = finp.tile([1, C], f32)
            nc.vector.tensor_mul(auxc[:], ce10[:], rinv[:])

            nuq = finp.tile([1, 1], f32)
            nc.vector.reduce_sum(nuq[:], mask[:], axis=AX.X)
            ninv = finp.tile([1, 1], f32)
            nc.vector.reciprocal(ninv[:], nuq[:])
            sreg = finp.tile([1, 1], f32)
            nc.vector.reduce_sum(sreg[:], regc[:], axis=AX.X)
            saux = finp.tile([1, 1], f32)
            nc.vector.reduce_sum(saux[:], auxc[:], axis=AX.X)

            lm_t = finp.tile([1, 1], f32)
            nc.sync.dma_start(lm_t[:], lm_d[:])
            t1 = finp.tile([1, 1], f32)
            nc.vector.tensor_mul(t1[:], sreg[:], lm_t[:])
            t2 = finp.tile([1, 1], f32)
            nc.vector.tensor_add(t2[:], t1[:], saux[:])
            t3 = finp.tile([1, 1], f32)
            nc.vector.tensor_mul(t3[:], t2[:], ninv[:])
            # fold in 0*warm so the warm-up collective isn't dead code
            res = finp.tile([1, 1], f32)
            nc.vector.scalar_tensor_tensor(
                res[:], wres[:], 0.0, t3[:], ALU.mult, ALU.add
            )
            nc.sync.dma_start(out_d[:], res[:])

    nc.compile()
    return nc


_NC_CACHE = {}


def _get_nc(rows_per_core=ROWS_PER_CORE, w=64, act_batch=4):
    key = (rows_per_core, w, act_batch)
    if key not in _NC_CACHE:
        _NC_CACHE[key] = build(rows_per_core, w, act_batch)
    return _NC_CACHE[key]


def make_in_maps(yhat, yg, u_zg, lmbd, rows_per_core=ROWS_PER_CORE):
    yhat = np.ascontiguousarray(np.asarray(yhat).astype(np.int32))
    yg = np.ascontiguousarray(np.asarray(yg, dtype=np.float32))
    u_zg = np.ascontiguousarray(np.asarray(u_zg, dtype=np.float32))
    lmbd = np.asarray(lmbd, dtype=np.float32).reshape(1, 1)
    n = yhat.shape[0]
    assert n == rows_per_core * N_CORES
    in_maps = []
    for i in range(N_CORES):
        s = slice(i * rows_per_core, (i + 1) * rows_per_core)
        in_maps.append(
            {"yhat": yhat[s], "yg": yg[s], "u_zg": u_zg[s], "lmbd": lmbd}
        )
    return in_maps


def run(yhat, yg, u_zg, lmbd, trace=False, rows_per_core=ROWS_PER_CORE, w=64,
        act_batch=4):
    from concourse import bass_utils

    nc = _get_nc(rows_per_core, w, act_batch)
    in_maps = make_in_maps(yhat, yg, u_zg, lmbd, rows_per_core)
    res = bass_utils.run_bass_kernel_spmd(
        nc, in_maps, core_ids=list(range(N_CORES)), trace=trace
    )
    val = np.float32(np.asarray(res.results[0]["out"]).reshape(())[()])
    return val, res


def kernel(yhat, yg, u_zg, lmbd):
    val, _ = run(yhat, yg, u_zg, lmbd)
    return np.asarray(val, dtype=np.float32).reshape(())



# revision 2
# speedup vs baseline: 1.3517x; 1.3517x over previous
"""Trainium2 Bass kernel for nn_AuxLoss (aux CE loss + erf regularizer, segment-
mean over K=10 classes), data-parallel over 8 NeuronCores.

Math (per reference):
  f(u)      = 0.5 - 0.5*erf((-0.5 - u)/(sigma*sqrt2)) = 0.5 + 0.5*erf((u+0.5)*sqrt2)
  row_reg_n = sum_d f(u[n,d])
  row_ce_n  = logsumexp(yg[n,:]) - yg[n, yhat[n]]
  per-class means over rows with yhat==k, averaged over present classes:
  out = mean_k(seg_ce/cnt) + lmbd * mean_k(seg_reg/(cnt*D))

Device strategy per core (131072 rows = 128 partitions x 1024 contiguous rows):
  - DMA: sync (HWDGE) ring carries yg halves 0-1, yhat, u even chunks; gpsimd
    (SWDGE) ring carries yg halves 2-3 and u odd chunks. yg rides at the head
    of both rings so logsumexp is ready before the first work tile; the 16 SDMA
    engines stay saturated (~340 GB/s) start to finish.
  - ACT does exactly 3 table loads: exp over the 4 yg quarters (bf16 out),
    one ln over the whole [128,1024] sumexp, then 16 uninterrupted erf chunks
    tracking the u stream (explicit chain deps keep the order).
  - per u chunk one bf16 work tile [128, 64, 76]:
      cols 0:64  erf(sqrt2*u + sqrt2/2)  (the 0.5+0.5* affine is folded into
                 the host fixup: seg_f = 0.5*D*cnt + 0.5*seg_erf)
      cols 64:74 onehot*yg   (diagonal trick: row-sum of the segment matmul
                 block = seg of yg[n,yhat[n]])
      col  74    ones        (counts)
      col  75    lse         (segment-sum of logsumexp via the same matmul)
    onehot[p,r,c] = (yhat==c) via iota compare (DVE, bf16)
  - PE: per 128-row group one matmul: onehot[:,g,:] stationary (128x10),
    work[:,g,:] moving (128x76), accumulating PSUM [10,76] over 1024 groups.
  - Each core DMAs its raw [10,76] PSUM partials out (3 KB); the host gathers
    the 8 partials and finishes the ~40-flop reduction in float64. No
    collectives, no on-device epilogue.
"""

import math
import sys

if "/opt/trn_rl_repo" not in sys.path:
    sys.path.insert(0, "/opt/trn_rl_repo")

import numpy as np

N_CORES = 8
N_FULL = 1048576
C = 10
D = 64
P = 128
ROWS_PER_CORE = N_FULL // N_CORES  # 131072
RPP = ROWS_PER_CORE // P  # 1024 rows per partition
SQ2 = math.sqrt(2.0)
W = 64  # u rows/partition per chunk
NCH = RPP // W  # 16
YW = 256  # yg rows/partition per chunk
NYG = RPP // YW  # 4
W_COLS = D + C + 2  # erf block | onehot*yg block | ones | lse


def build():
    from concourse import bacc, mybir, tile
    from concourse.tile_rust import add_dep_helper

    f32 = mybir.dt.float32
    bf16 = mybir.dt.bfloat16
    i32 = mybir.dt.int32
    FT = mybir.ActivationFunctionType
    ALU = mybir.AluOpType
    AX = mybir.AxisListType

    nc = bacc.Bacc(
        "TRN2", target_bir_lowering=False, debug=False, num_devices=N_CORES
    )

    yh_d = nc.dram_tensor("yhat", [ROWS_PER_CORE], i32, kind="ExternalInput")
    yg_d = nc.dram_tensor("yg", [ROWS_PER_CORE, C], f32, kind="ExternalInput")
    u_d = nc.dram_tensor("u_zg", [ROWS_PER_CORE, D], f32, kind="ExternalInput")
    out_d = nc.dram_tensor("out", [C, W_COLS], f32, kind="ExternalOutput")

    u_v = u_d[:].rearrange("(p r) d -> p r d", p=P)
    yg_v = yg_d[:].rearrange("(p r) c -> p r c", p=P)
    yh_v = yh_d[:].rearrange("(p r) -> p r", p=P)

    last_act = [None]

    def act_ordered(*args, **kwargs):
        """scalar.activation with an explicit chain dep so the Tile scheduler
        cannot interleave ACT functions (each interleave costs a ~1.3 us
        ACT table-set load)."""
        inst = nc.scalar.activation(*args, **kwargs)
        raw = getattr(inst, "ins", inst)
        if last_act[0] is not None:
            add_dep_helper(raw, last_act[0], sync=True, reason="act set order")
        last_act[0] = raw
        return inst

    def chain(track, inst, why):
        """Force same-ring DMA trigger order (ring drains FIFO per engine)."""
        raw = getattr(inst, "ins", inst)
        if track[0] is not None:
            add_dep_helper(raw, track[0], sync=True, reason=why)
        track[0] = raw
        return inst

    last_sync = [None]
    last_gps = [None]

    with tile.TileContext(nc) as tc:
        with (
            tc.tile_pool(name="const", bufs=1) as constp,
            tc.tile_pool(name="io", bufs=1) as iop,
            tc.tile_pool(name="work", bufs=1) as workp,
            tc.tile_pool(name="psum", bufs=1, space="PSUM") as psump,
        ):
            # --- constants ---
            erf_bias = constp.tile([P, 1], f32)
            nc.vector.memset(erf_bias[:], 0.5 * SQ2)
            iota_f = constp.tile([P, 1, C], f32)
            nc.gpsimd.iota(
                iota_f[:, 0, :], [[1, C]],
                channel_multiplier=0, allow_small_or_imprecise_dtypes=True,
            )

            # --- DMA program: yg at the head of both rings, then u ---
            yg_all = constp.tile([P, RPP, C], f32)
            chain(last_sync, nc.sync.dma_start(
                yg_all[:, 0 * YW:1 * YW, :], yg_v[:, 0 * YW:1 * YW, :]), "syncq")
            chain(last_sync, nc.sync.dma_start(
                yg_all[:, 1 * YW:2 * YW, :], yg_v[:, 1 * YW:2 * YW, :]), "syncq")
            yh_i = constp.tile([P, RPP], i32)
            chain(last_sync, nc.sync.dma_start(yh_i[:], yh_v), "syncq")
            chain(last_gps, nc.gpsimd.dma_start(
                yg_all[:, 2 * YW:3 * YW, :], yg_v[:, 2 * YW:3 * YW, :]), "gpsq")
            chain(last_gps, nc.gpsimd.dma_start(
                yg_all[:, 3 * YW:4 * YW, :], yg_v[:, 3 * YW:4 * YW, :]), "gpsq")

            yh_f = constp.tile([P, RPP], f32)
            nc.vector.tensor_copy(yh_f[:], yh_i[:])

            u_ts = {}
            for ci in range(NCH):
                u_t = iop.tile([P, W, D], f32, name="u_t", bufs=5)
                if ci % 2 == 0:
                    chain(last_sync,
                          nc.sync.dma_start(u_t[:], u_v[:, ci * W:(ci + 1) * W, :]),
                          "syncq")
                else:
                    chain(last_gps,
                          nc.gpsimd.dma_start(u_t[:], u_v[:, ci * W:(ci + 1) * W, :]),
                          "gpsq")
                u_ts[ci] = u_t

            # --- ACT: exp chunks -> ln -> erf chunks (3 table loads total) ---
            sume = constp.tile([P, RPP], f32)
            for yi in range(NYG):
                ex_t = workp.tile([P, YW, C], bf16, name="ex_t", bufs=2)
                act_ordered(ex_t[:], yg_all[:, yi * YW:(yi + 1) * YW, :], FT.Exp)
                nc.vector.reduce_sum(sume[:, yi * YW:(yi + 1) * YW], ex_t[:], axis=AX.X)
            lse = constp.tile([P, RPP], f32)
            act_ordered(lse[:], sume[:], FT.Ln)

            # --- streamed erf + work-tile assembly + segment matmuls ---
            ps = psump.tile([C, W_COLS], f32)
            for ci in range(NCH):
                r0, r1 = ci * W, (ci + 1) * W
                work_t = workp.tile([P, W, W_COLS], bf16, name="work_t", bufs=4)
                act_ordered(
                    work_t[:, :, 0:D], u_ts.pop(ci)[:], FT.Erf,
                    bias=erf_bias[:], scale=SQ2,
                )
                oh_t = workp.tile([P, W, C], bf16, name="oh_t", bufs=4)
                nc.vector.tensor_tensor(
                    oh_t[:],
                    yh_f[:, r0:r1].broadcast_to([P, W, C]),
                    iota_f[:].broadcast_to([P, W, C]),
                    ALU.is_equal,
                )
                nc.vector.tensor_tensor(
                    work_t[:, :, D:D + C], oh_t[:], yg_all[:, r0:r1, :], ALU.mult
                )
                nc.vector.memset(work_t[:, :, D + C], 1.0)
                nc.vector.tensor_copy(work_t[:, :, D + C + 1], lse[:, r0:r1])
                for g in range(W):
                    nc.tensor.matmul(
                        ps[:], oh_t[:, g, :], work_t[:, g, :],
                        start=(ci == 0 and g == 0),
                        stop=(ci == NCH - 1 and g == W - 1),
                    )

            # --- raw partials out; host finishes the 40-flop epilogue ---
            acc = constp.tile([C, W_COLS], f32)
            nc.vector.tensor_copy(acc[:], ps[:])
            chain(last_sync, nc.sync.dma_start(out_d[:], acc[:]), "syncq")

    nc.compile()
    return nc


_NC_CACHE = {}


def _get_nc():
    if "nc" not in _NC_CACHE:
        _NC_CACHE["nc"] = build()
    return _NC_CACHE["nc"]


def make_in_maps(yhat, yg, u_zg):
    yhat = np.ascontiguousarray(np.asarray(yhat).astype(np.int32))
    yg = np.ascontiguousarray(np.asarray(yg, dtype=np.float32))
    u_zg = np.ascontiguousarray(np.asarray(u_zg, dtype=np.float32))
    n = yhat.shape[0]
    assert n == ROWS_PER_CORE * N_CORES
    in_maps = []
    for i in range(N_CORES):
        s = slice(i * ROWS_PER_CORE, (i + 1) * ROWS_PER_CORE)
        in_maps.append({"yhat": yhat[s], "yg": yg[s], "u_zg": u_zg[s]})
    return in_maps


def _finalize(parts, lmbd):
    """Gather the 8 per-core [10,76] partials and finish in float64.
    cols 0:64 seg erf | 64:74 seg onehot*yg | 74 counts | 75 seg lse."""
    s = np.zeros((C, W_COLS), dtype=np.float64)
    for p in parts:
        s += np.asarray(p, dtype=np.float64)
    seg_erf = s[:, 0:D].sum(axis=1)
    picked = s[:, D:D + C].sum(axis=1)
    cnt = s[:, D + C]
    seg_lse = s[:, D + C + 1]
    present = cnt > 0
    denom = np.where(present, cnt, 1.0)
    # f(u) = 0.5 + 0.5*erf((u+0.5)*sqrt2): seg_f = 0.5*D*cnt + 0.5*seg_erf
    reg_c = (0.5 * D * cnt + 0.5 * seg_erf) / (denom * D)
    aux_c = (seg_lse - picked) / denom
    nuq = present.sum()
    val = (
        np.where(present, aux_c, 0.0).sum()
        + float(np.asarray(lmbd).reshape(())) * np.where(present, reg_c, 0.0).sum()
    ) / nuq
    return np.float32(val)


def run(yhat, yg, u_zg, lmbd, trace=False):
    from concourse import bass_utils

    nc = _get_nc()
    in_maps = make_in_maps(yhat, yg, u_zg)
    res = bass_utils.run_bass_kernel_spmd(
        nc, in_maps, core_ids=list(range(N_CORES)), trace=trace
    )
    parts = [res.results[i]["out"] for i in range(N_CORES)]
    return _finalize(parts, lmbd), res


def kernel(yhat, yg, u_zg, lmbd):
    val, _ = run(yhat, yg, u_zg, lmbd)
    return np.asarray(val, dtype=np.float32).reshape(())
